# revision 1
# baseline (speedup 1.0000x reference)
"""Trainium2 Bass kernel for nn_JointPairHead: edge gather + LN + 3x(Linear->BN->ReLU) -> logits.

v2 design:
- dma_gather (SWDGE Ant, <=1024 idxs/op) fetches bf16 x rows in one op per
  1024 edges instead of 8 indirect DMAs per 512 -> ~7x less gpsimd time.
- int16 gather indices: nodes split lo/hi at 32766 with separate gather base
  addresses; edges bucketed [LL, LH, HL, HH] per core (host-side permutation),
  buckets padded to 512 with dummy edges pointing at a zero row.
- z kept resident in SBUF as bf16 (no DRAM spills): saves ~192MB HBM/core.
- BN stats: accum_out on the PSUM->SBUF copy (sum) + DVE square pass (sumsq);
  dummy-edge contamination removed analytically after the AllReduce.
- warmup AllReduce at t=0 hides the ~180us first-collective setup.
"""

import numpy as np

N_NODES = 50000
D = 256
E_TOT = 262144
NCORES = 8
ESH = E_TOT // NCORES          # 32768 real edges per core
EBLK = 512
EPS = 1e-5
NL = 3
LOTH = 32766                   # nodes < LOTH gather via lo base (idx=node+1)
HIOFF = 32767                  # hi base starts at padded-x row 32767
ZHI = 17234                    # zero row (abs 50001) relative to hi base
XROWS = N_NODES + 2


def _rne_bf16(a):
    u = np.ascontiguousarray(a, dtype=np.float32).view(np.uint32)
    return ((u + 0x7FFF + ((u >> 16) & 1)) >> 16).astype(np.uint16)


def build_nc(caps, num_devices=NCORES):
    import concourse.bass as bass
    import concourse.mybir as mybir
    import concourse.tile as tile
    from concourse import bacc
    from concourse.masks import make_identity

    f32 = mybir.dt.float32
    bf16 = mybir.dt.bfloat16
    i16 = mybir.dt.int16
    A = mybir.ActivationFunctionType
    ALU = mybir.AluOpType
    AX = mybir.AxisListType

    T = sum(caps)
    assert T % 1024 == 0
    NBLK = T // EBLK
    SBLK = 1024
    NS = T // SBLK
    B1 = caps[0]
    B2 = caps[0] + caps[1]
    B3 = caps[0] + caps[1] + caps[2]
    nd_glob = float((T - ESH) * num_devices)

    # gather pieces: [start, len, src_hi, dst_hi]; len in {512, 1024}
    bnds = sorted({0, B1, B2, B3, T})
    pieces = []
    for a, b in zip(bnds, bnds[1:]):
        p = a
        while p < b:
            L = min(1024, b - p)
            pieces.append((p, L, p >= B2, (B1 <= p < B2) or (p >= B3)))
            p += L

    groups = [list(range(num_devices))]
    inv_d = 1.0 / D
    inv_e = 1.0 / E_TOT

    nc = bacc.Bacc("TRN2", target_bir_lowering=False, debug=False,
                   num_devices=num_devices, num_swdge_queues=4)

    xb = nc.dram_tensor("xb", [XROWS, D], bf16, kind="ExternalInput").ap()
    srci = nc.dram_tensor("srci", [128, T // 16], i16, kind="ExternalInput").ap()
    dsti = nc.dram_tensor("dsti", [128, T // 16], i16, kind="ExternalInput").ap()
    wts = [nc.dram_tensor(f"w{i}t", [D, D], bf16, kind="ExternalInput").ap()
           for i in range(NL)]
    wot = nc.dram_tensor("wot", [D, 1], bf16, kind="ExternalInput").ap()
    gam = nc.dram_tensor("gam", [NL, D], f32, kind="ExternalInput").ap()
    bet = nc.dram_tensor("bet", [NL, D], f32, kind="ExternalInput").ap()
    out = nc.dram_tensor("out", [T], f32, kind="ExternalOutput").ap()

    cc_space = "Shared" if num_devices > 4 else "Local"
    ccin = [nc.dram_tensor(f"ccin{i}", [128, 4], f32, kind="Internal").ap()
            for i in range(NL + 1)]
    ccout = [nc.dram_tensor(f"ccout{i}", [128, 4], f32, kind="Internal",
                            addr_space=cc_space).ap()
             for i in range(NL + 1)]

    with tile.TileContext(nc) as tc:
        with (
            tc.tile_pool(name="const", bufs=1) as cpool,
            tc.tile_pool(name="gather", bufs=2) as gp,
            tc.tile_pool(name="work", bufs=2) as wp,
            tc.tile_pool(name="stats", bufs=1) as sp,
        ):
            # ---- warmup collective: hide first-CC setup under phase 0 ----
            ccw = sp.tile([128, 4], f32, name="ccw")
            nc.vector.memset(ccw[:], 0.0)
            nc.sync.dma_start(out=ccin[NL][:, :], in_=ccw[:])
            if num_devices == 1:
                nc.sync.dma_start(out=ccout[NL][:, :], in_=ccin[NL][:, :])
            else:
                nc.gpsimd.collective_compute(
                    "AllReduce", ALU.add, replica_groups=groups,
                    ins=[ccin[NL][:, :]], outs=[ccout[NL][:, :]])

            # ---- constants ----
            identf = cpool.tile([128, 128], f32, name="identf")
            make_identity(nc, identf[:])
            identb = cpool.tile([128, 128], bf16, name="identb")
            nc.scalar.copy(out=identb[:], in_=identf[:])
            isrc = cpool.tile([128, T // 16], i16, name="isrc")
            idst = cpool.tile([128, T // 16], i16, name="idst")
            nc.sync.dma_start(out=isrc[:], in_=srci[:])
            nc.sync.dma_start(out=idst[:], in_=dsti[:])
            wsb = []
            for i in range(NL):
                chunks = []
                for c in range(2):
                    t = cpool.tile([128, D], bf16, name=f"w{i}c{c}")
                    nc.sync.dma_start(out=t[:], in_=wts[i][c * 128:(c + 1) * 128, :])
                    chunks.append(t)
                wsb.append(chunks)
            wot_sb = []
            for c in range(2):
                t = cpool.tile([128, 1], bf16, name=f"wo{c}")
                nc.sync.dma_start(out=t[:], in_=wot[c * 128:(c + 1) * 128, :])
                wot_sb.append(t)
            gam_sb, bet_sb = [], []
            for i in range(NL):
                g = cpool.tile([128, 2], f32, name=f"gam{i}")
                b = cpool.tile([128, 2], f32, name=f"bet{i}")
                for c in range(2):
                    nc.sync.dma_start(out=g[:, c:c + 1],
                                      in_=gam[i, c * 128:(c + 1) * 128])
                    nc.sync.dma_start(out=b[:, c:c + 1],
                                      in_=bet[i, c * 128:(c + 1) * 128])
                gam_sb.append(g)
                bet_sb.append(b)

            # ---- persistent state ----
            zsb = [cpool.tile([128, T], bf16, name=f"zsb{j}") for j in range(2)]
            Sac = [[sp.tile([128, NBLK if i == 0 else NS], f32,
                            name=f"S{i}_{j}") for j in range(2)]
                   for i in range(NL)]
            SSac = [[sp.tile([128, NBLK if i == 0 else NS], f32,
                             name=f"SS{i}_{j}") for j in range(2)]
                    for i in range(NL)]
            a_ab = [sp.tile([128, 2], f32, name=f"a{i}") for i in range(NL)]
            b_ab = [sp.tile([128, 2], f32, name=f"b{i}") for i in range(NL)]
            vsb = [sp.tile([128, 2], bf16, name=f"v{i}") for i in range(NL)]
            zero2 = sp.tile([128, 2], bf16, name="zero2")
            nc.vector.memset(zero2[:], 0.0)

            def produce_z0(pp, blk, rhs):
                """phase0: z0 = W0 @ rhs; ACT-heavy split (DVE is busy)."""
                li = 0
                sl = slice(blk * EBLK, (blk + 1) * EBLK)
                for j in range(2):
                    zps = pp.tile([128, EBLK], f32, name=f"zps{j}", tag=f"zps{j}")
                    for c in range(2):
                        nc.tensor.matmul(
                            out=zps[:],
                            lhsT=wsb[li][c][:, j * 128:(j + 1) * 128],
                            rhs=rhs[c][:], start=(c == 0), stop=(c == 1))
                    dst = zsb[j][:, sl]
                    nc.scalar.activation(
                        out=dst, in_=zps[:], func=A.Copy,
                        accum_out=Sac[li][j][:, blk:blk + 1])
                    scr = wp.tile([128, EBLK], bf16, name=f"scr{j}", tag="scr")
                    nc.vector.scalar_tensor_tensor(
                        out=scr[:], in0=dst, scalar=1.0, in1=dst,
                        op0=ALU.mult, op1=ALU.mult,
                        accum_out=SSac[li][j][:, blk:blk + 1])

            # ================= Phase 0: gather + LN + layer 0 =================
            pp0_cm = tc.tile_pool(name="psum0", bufs=2, space="PSUM")
            pp0 = pp0_cm.__enter__()
            for pidx, (p0, L, shi, dhi) in enumerate(pieces):
                k = L // 128
                nb = L // EBLK
                cols = slice(p0 // 16, (p0 + L) // 16)
                xs = gp.tile([128, 8, D], bf16, name="xs", tag="xs")
                xd = gp.tile([128, 8, D], bf16, name="xd", tag="xd")
                qn = (2 * pidx) % 4
                nc.gpsimd.dma_gather(
                    out_ap=xs[:, 0:k, :],
                    in_ap=(xb[HIOFF:, :] if shi else xb[:, :]),
                    idxs_ap=isrc[:, cols], num_idxs=L, num_idxs_reg=L,
                    elem_size=D, queue_num=qn)
                nc.gpsimd.dma_gather(
                    out_ap=xd[:, 0:k, :],
                    in_ap=(xb[HIOFF:, :] if dhi else xb[:, :]),
                    idxs_ap=idst[:, cols], num_idxs=L, num_idxs_reg=L,
                    elem_size=D, queue_num=qn + 1)
                for b in range(nb):
                    blk = p0 // EBLK + b
                    g0 = b * 4
                    h = wp.tile([128, 4, D], bf16, name="h", tag="h")
                    nc.vector.tensor_add(
                        out=h[:].rearrange("p a b -> p (a b)"),
                        in0=xs[:, g0:g0 + 4, :].rearrange("p a b -> p (a b)"),
                        in1=xd[:, g0:g0 + 4, :].rearrange("p a b -> p (a b)"))
                    hsq = wp.tile([128, 4, D], bf16, name="hsq", tag="hsq")
                    nc.vector.tensor_mul(
                        out=hsq[:].rearrange("p a b -> p (a b)"),
                        in0=h[:].rearrange("p a b -> p (a b)"),
                        in1=h[:].rearrange("p a b -> p (a b)"))
                    Sln = wp.tile([128, 4], f32, name="Sln", tag="Sln")
                    SSln = wp.tile([128, 4], f32, name="SSln", tag="SSln")
                    nc.vector.reduce_sum(out=Sln[:], in_=h[:], axis=AX.X)
                    nc.vector.reduce_sum(out=SSln[:], in_=hsq[:], axis=AX.X)
                    mu2 = wp.tile([128, 4], f32, name="mu2", tag="mu2")
                    var = wp.tile([128, 4], f32, name="var", tag="var")
                    inv = wp.tile([128, 4], f32, name="inv", tag="inv")
                    rs = wp.tile([128, 4], f32, name="rs", tag="rs")
                    bneg = wp.tile([128, 4], f32, name="bneg", tag="bneg")
                    nc.scalar.activation(out=mu2[:], in_=Sln[:], func=A.Square,
                                         scale=inv_d)
                    nc.vector.scalar_tensor_tensor(
                        out=var[:], in0=SSln[:], scalar=inv_d, in1=mu2[:],
                        op0=ALU.mult, op1=ALU.subtract)
                    nc.vector.tensor_scalar_add(out=var[:], in0=var[:],
                                                scalar1=EPS)
                    nc.vector.reciprocal(out=inv[:], in_=var[:])
                    nc.scalar.sqrt(out=rs[:], in_=inv[:])
                    nc.vector.scalar_tensor_tensor(
                        out=bneg[:], in0=Sln[:], scalar=-inv_d, in1=rs[:],
                        op0=ALU.mult, op1=ALU.mult)
                    hn = wp.tile([128, 4, D], bf16, name="hn", tag="hn")
                    for g in range(4):
                        nc.scalar.activation(
                            out=hn[:, g, :], in_=h[:, g, :], func=A.Identity,
                            bias=bneg[:, g:g + 1], scale=rs[:, g:g + 1])
                    ht = []
                    for c in range(2):
                        tp = pp0.tile([128, EBLK], bf16, name=f"tp{c}",
                                      tag=f"tp{c}")
                        for g in range(4):
                            nc.tensor.transpose(
                                out=tp[:, g * 128:(g + 1) * 128],
                                in_=hn[:, g, c * 128:(c + 1) * 128],
                                identity=identb[:])
                        hc = wp.tile([128, EBLK], bf16, name=f"ht{c}",
                                     tag=f"ht{c}")
                        nc.scalar.copy(out=hc[:], in_=tp[:])
                        ht.append(hc)
                    produce_z0(pp0, blk, ht)

            # ============ stats finalize + BN affine ============
            def finalize(li):
                st4 = sp.tile([128, 4], f32, name=f"st4_{li}")
                for j in range(2):
                    nc.vector.reduce_sum(out=st4[:, j:j + 1], in_=Sac[li][j][:],
                                         axis=AX.X)
                    nc.vector.reduce_sum(out=st4[:, 2 + j:3 + j],
                                         in_=SSac[li][j][:], axis=AX.X)
                nc.sync.dma_start(out=ccin[li][:, :], in_=st4[:])
                if num_devices == 1:
                    nc.sync.dma_start(out=ccout[li][:, :], in_=ccin[li][:, :])
                else:
                    nc.gpsimd.collective_compute(
                        "AllReduce", ALU.add, replica_groups=groups,
                        ins=[ccin[li][:, :]], outs=[ccout[li][:, :]])
                gst = sp.tile([128, 4], f32, name=f"gst{li}")
                nc.sync.dma_start(out=gst[:], in_=ccout[li][:, :])
                if li > 0:
                    # remove dummy-edge contribution: S -= nd*v, SS -= nd*v^2
                    vf = sp.tile([128, 2], f32, name=f"vf{li}")
                    vq = sp.tile([128, 2], f32, name=f"vq{li}")
                    nc.vector.tensor_scalar_mul(out=vf[:], in0=vsb[li][:],
                                                scalar1=1.0)
                    nc.vector.tensor_mul(out=vq[:], in0=vf[:], in1=vf[:])
                    nc.vector.scalar_tensor_tensor(
                        out=gst[:, 0:2], in0=vf[:], scalar=-nd_glob,
                        in1=gst[:, 0:2], op0=ALU.mult, op1=ALU.add)
                    nc.vector.scalar_tensor_tensor(
                        out=gst[:, 2:4], in0=vq[:], scalar=-nd_glob,
                        in1=gst[:, 2:4], op0=ALU.mult, op1=ALU.add)
                bmu = sp.tile([128, 2], f32, name=f"bmu{li}")
                bmu2 = sp.tile([128, 2], f32, name=f"bmu2{li}")
                bvar = sp.tile([128, 2], f32, name=f"bvar{li}")
                binv = sp.tile([128, 2], f32, name=f"binv{li}")
                brs = sp.tile([128, 2], f32, name=f"brs{li}")
                tt = sp.tile([128, 2], f32, name=f"tt{li}")
                nc.scalar.mul(out=bmu[:], in_=gst[:, 0:2], mul=inv_e)
                nc.scalar.square(out=bmu2[:], in_=bmu[:])
                nc.vector.scalar_tensor_tensor(
                    out=bvar[:], in0=gst[:, 2:4], scalar=inv_e, in1=bmu2[:],
                    op0=ALU.mult, op1=ALU.subtract)
                nc.vector.tensor_scalar_add(out=bvar[:], in0=bvar[:],
                                            scalar1=EPS)
                nc.vector.reciprocal(out=binv[:], in_=bvar[:])
                nc.scalar.sqrt(out=brs[:], in_=binv[:])
                nc.vector.tensor_mul(out=a_ab[li][:], in0=gam_sb[li][:],
                                     in1=brs[:])
                nc.vector.tensor_mul(out=tt[:], in0=a_ab[li][:], in1=bmu[:])
                nc.vector.tensor_sub(out=b_ab[li][:], in0=bet_sb[li][:],
                                     in1=tt[:])
                if li + 1 < NL:
                    # dummy z of next layer: v_{li+1} = W_{li+1} @ relu(a*vprev+b)
                    vprev = zero2 if li == 0 else vsb[li]
                    hd = sp.tile([128, 2], bf16, name=f"hd{li}")
                    for j in range(2):
                        nc.scalar.activation(
                            out=hd[:, j:j + 1], in_=vprev[:, j:j + 1],
                            func=A.Relu, bias=b_ab[li][:, j:j + 1],
                            scale=a_ab[li][:, j:j + 1])
                    for j in range(2):
                        vps = pp1.tile([128, SBLK], f32, name="vps", tag="zpw0")
                        for c in range(2):
                            nc.tensor.matmul(
                                out=vps[:, 0:1],
                                lhsT=wsb[li + 1][c][:, j * 128:(j + 1) * 128],
                                rhs=hd[:, c:c + 1],
                                start=(c == 0), stop=(c == 1))
                        nc.scalar.copy(out=vsb[li + 1][:, j:j + 1],
                                       in_=vps[:, 0:1])

            pp0_cm.__exit__(None, None, None)
            pp1_cm = tc.tile_pool(name="psum1", bufs=2, space="PSUM")
            pp1 = pp1_cm.__enter__()
            finalize(0)

            # ================= Layers 1..2 (1024-wide superblocks) ============
            for li in range(1, NL):
                for sb in range(NS):
                    sl = slice(sb * SBLK, (sb + 1) * SBLK)
                    rh = []
                    for j in range(2):
                        hc = wp.tile([128, SBLK], bf16, name=f"rh{j}",
                                     tag=f"rh{j}")
                        nc.scalar.activation(
                            out=hc[:], in_=zsb[j][:, sl], func=A.Relu,
                            bias=b_ab[li - 1][:, j:j + 1],
                            scale=a_ab[li - 1][:, j:j + 1])
                        rh.append(hc)
                    for j in range(2):
                        zpw = pp1.tile([128, SBLK], f32, name=f"zpw{j}",
                                       tag=f"zpw{j}")
                        for half in range(2):
                            hs = slice(half * EBLK, (half + 1) * EBLK)
                            for c in range(2):
                                nc.tensor.matmul(
                                    out=zpw[:, hs],
                                    lhsT=wsb[li][c][:, j * 128:(j + 1) * 128],
                                    rhs=rh[c][:, hs],
                                    start=(c == 0), stop=(c == 1))
                        dst = zsb[j][:, sl]
                        if j == 0:
                            nc.scalar.activation(
                                out=dst, in_=zpw[:], func=A.Copy,
                                accum_out=Sac[li][j][:, sb:sb + 1])
                        else:
                            nc.vector.tensor_scalar(
                                out=dst, in0=zpw[:], scalar1=1.0, scalar2=0.0,
                                op0=ALU.mult, op1=ALU.add,
                                accum_out=Sac[li][j][:, sb:sb + 1])
                        scr = wp.tile([128, SBLK], bf16, name=f"scw{j}",
                                      tag="scw")
                        nc.vector.scalar_tensor_tensor(
                            out=scr[:], in0=dst, scalar=1.0, in1=dst,
                            op0=ALU.mult, op1=ALU.mult,
                            accum_out=SSac[li][j][:, sb:sb + 1])
                finalize(li)

            # ================= Phase 3: final projection (1024-wide) ==========
            for sb in range(NS):
                sl = slice(sb * SBLK, (sb + 1) * SBLK)
                lpsf = pp1.tile([128, SBLK], f32, name="lps", tag="zpw1")
                fh = []
                for j in range(2):
                    hc = wp.tile([128, SBLK], bf16, name=f"fh{j}", tag=f"rh{j}")
                    nc.scalar.activation(
                        out=hc[:], in_=zsb[j][:, sl], func=A.Relu,
                        bias=b_ab[NL - 1][:, j:j + 1],
                        scale=a_ab[NL - 1][:, j:j + 1])
                    fh.append(hc)
                for half in range(2):
                    hs = slice(half * EBLK, (half + 1) * EBLK)
                    for j in range(2):
                        nc.tensor.matmul(out=lpsf[0:1, hs],
                                         lhsT=wot_sb[j][:], rhs=fh[j][:, hs],
                                         start=(j == 0), stop=(j == 1))
                lsb = wp.tile([1, SBLK], f32, name="lsb", tag="lsb")
                nc.vector.tensor_scalar_mul(out=lsb[:], in0=lpsf[0:1, :],
                                            scalar1=1.0)
                nc.sync.dma_start(out=out[sb * SBLK:(sb + 1) * SBLK],
                                  in_=lsb[:])
            pp1_cm.__exit__(None, None, None)

    nc.compile()
    return nc


_NC = None
_NC_KEY = None


def _prep_core(src, dst, caps):
    """Bucket one core's edges into [LL, LH, HL, HH] regions with shared caps.
    Returns (srcv, dstv, pos) int16 index arrays + real-edge positions."""
    T = sum(caps)
    shi = src >= LOTH
    dhi = dst >= LOTH
    bucket = (shi.astype(np.int64) << 1) | dhi.astype(np.int64)
    off = np.concatenate([[0], np.cumsum(caps)[:-1]])
    order = np.argsort(bucket, kind="stable")
    # rank within bucket
    pos = np.empty(len(src), dtype=np.int64)
    counts = np.bincount(bucket, minlength=4)
    start = 0
    for q in range(4):
        n = counts[q]
        pos[order[start:start + n]] = off[q] + np.arange(n)
        start += n
    B1, B2, B3 = caps[0], caps[0] + caps[1], caps[0] + caps[1] + caps[2]
    p = np.arange(T)
    s_hi_region = p >= B2
    d_hi_region = ((p >= B1) & (p < B2)) | (p >= B3)
    srcv = np.where(s_hi_region, ZHI, 0).astype(np.int16)
    dstv = np.where(d_hi_region, ZHI, 0).astype(np.int16)
    srcv[pos] = np.where(shi, src + 1 - HIOFF, src + 1).astype(np.int16)
    dstv[pos] = np.where(dhi, dst + 1 - HIOFF, dst + 1).astype(np.int16)
    return srcv, dstv, pos


def _pack16(a):
    return np.tile(np.ascontiguousarray(a.reshape(-1, 16).T), (8, 1))


def kernel(**inputs):
    global _NC, _NC_KEY
    from concourse import bass_utils
    import ml_dtypes

    x = np.ascontiguousarray(np.asarray(inputs["x"], dtype=np.float32))
    ei = np.asarray(inputs["jg_edge_index"])
    ln_w = np.asarray(inputs["ln_w"], dtype=np.float32)
    Ws = np.asarray(inputs["Ws"], dtype=np.float32)
    gammas = np.asarray(inputs["gammas"], dtype=np.float32)
    betas = np.asarray(inputs["betas"], dtype=np.float32)
    W_out = np.asarray(inputs["W_out"], dtype=np.float32)

    # padded bf16 x: [zero, x, zero]
    xbu = np.zeros((XROWS, D), dtype=np.uint16)
    xbu[1:N_NODES + 1] = _rne_bf16(x)
    xbv = xbu.view(ml_dtypes.bfloat16)

    # bucket counts per core -> shared caps
    srcs, dsts = [], []
    counts = np.zeros((NCORES, 4), dtype=np.int64)
    for c in range(NCORES):
        sl = slice(c * ESH, (c + 1) * ESH)
        src = np.asarray(ei[0, sl], dtype=np.int64)
        dst = np.asarray(ei[1, sl], dtype=np.int64)
        srcs.append(src)
        dsts.append(dst)
        b = ((src >= LOTH).astype(np.int64) << 1) | (dst >= LOTH)
        counts[c] = np.bincount(b, minlength=4)
    caps = [int(-(-counts[:, q].max() // EBLK) * EBLK) for q in range(4)]
    # pad total to a 1024 multiple for uniform gather pieces
    if sum(caps) % 1024:
        caps[3] += EBLK
    caps = tuple(caps)
    T = sum(caps)

    W0f = Ws[0] * ln_w[None, :]
    wts = [np.ascontiguousarray(W0f.T), np.ascontiguousarray(Ws[1].T),
           np.ascontiguousarray(Ws[2].T)]
    wts = [_rne_bf16(w).view(ml_dtypes.bfloat16) for w in wts]
    wot = _rne_bf16(np.ascontiguousarray(W_out.reshape(1, D).T)).view(
        ml_dtypes.bfloat16)

    if _NC is None or _NC_KEY != caps:
        _NC = build_nc(caps)
        _NC_KEY = caps

    in_maps, poss = [], []
    for c in range(NCORES):
        srcv, dstv, pos = _prep_core(srcs[c], dsts[c], caps)
        poss.append(pos)
        in_maps.append({
            "xb": xbv,
            "srci": _pack16(srcv),
            "dsti": _pack16(dstv),
            "w0t": wts[0], "w1t": wts[1], "w2t": wts[2],
            "wot": wot,
            "gam": gammas, "bet": betas,
        })
    global _last_in_maps
    _last_in_maps = in_maps
    res = bass_utils.run_bass_kernel_spmd(_NC, in_maps,
                                          core_ids=list(range(NCORES)))
    full = np.empty(E_TOT, dtype=np.float32)
    for c in range(NCORES):
        o = np.asarray(res.results[c]["out"])
        full[c * ESH:(c + 1) * ESH] = o[poss[c]]
    return full


_last_in_maps = None

